# revision 31
# baseline (speedup 1.0000x reference)
"""Trainium2 Bass kernel for the batched linear state-space model

    x_{k+1} = A x_k + B u_k ;  y_k = C x_k + D u_k   (y uses pre-update state)

Shapes: x0 [32,64,1], us [32,16384,64,1], A/B/C/D [64,64] -> y [32,16384,64,1].

Method (v3: fp8 DoubleRow tail taps)
------------------------------------
A is stable (spectral radius ~0.58), so the exact scan equals a causal FIR
with geometrically decaying matrix taps, truncated at P = 11 taps:

    y_k = sum_{i=0}^{P-1} V_i u''[k+i],   V_i = C A^{P-2-i} B (i<P-1), V_{P-1}=D

with u'' = [0...0, u_0 ... u_{N-1}] (P-1 zeros prepended).  The x0
transient sum_k C A^k x0 is added EXACTLY on the host for k < 32.

Polyphase S=2 layout as in v2: SBUF partitions 0:64 hold lo[m] = u''[2m],
64:128 hold hi[m] = u''[2m+1]; block G_j (128x128) at shift j combines taps
{V_{2j-1}, V_{2j}, V_{2j+1}} so that one [128,T] rhs pass yields T even and
T odd outputs.  P=11 gives 6 blocks G_0..G_5.

New in v3: G_0..G_3 carry only small taps (|CA^kB| ~ 0.58^k, k>=2), so they
run in fp8 e4m3 with MatmulPerfMode.DoubleRow: one PE pass contracts 256
rows (two 128-blocks at adjacent shifts) per column -- 2x MACs/cycle.  The
hardware accepts rhs APs whose two K-halves are the SAME fp8 image at
column shifts (j, j+1) (dim1 stride 1, verified on HW).  Dominant blocks
G_4, G_5 (taps D, CB, CAB, CA^2B) stay bf16.  Per 512-col tile: 2 bf16
passes + 2 DR passes = 4 PE passes vs 5 in v2 (and truncation error drops:
P 9 -> 11).  All taps are pre-scaled by 128 so the smallest fp8 blocks stay
in e4m3's normal range; the PSUM evacuation multiplies by 1/128 while
converting f32 -> bf16 (vector tensor_scalar / scalar activation, split
between both engines).

HW-verified constraints baked in here (each wedges the device if violated):
  * the FIRST tensor-engine instruction must be a non-DoubleRow matmul
    (hence the warmup matmul, forced first via high_priority + first DMA);
  * within one PSUM accumulation group, DR passes must come BEFORE bf16
    passes (bf16 -> DR transition with start=False is fatal);
  * no engine may WRITE fp8 to SBUF in a program containing DR matmuls
    (vector/scalar/gpsimd casts all crash), so the fp8 image is prepared on
    the HOST and DMA'd in (gpsimd queue) alongside the bf16 image (sync
    queue); output goes out on the scalar queue.  Per-core DMA: 12.6 MB in
    + 8.4 MB out.  DMA chunk sizes ramp 1,1,2,4,8 tiles at the start and
    down at the end as in v2.
"""
import numpy as np
from contextlib import ExitStack

# ---------------------------------------------------------------------------
# environment patches (this container's walrus encodes at most ONE semaphore
# wait per instruction; Tile emits more on the exit drain and on join points)
# ---------------------------------------------------------------------------
import orjson
import concourse.bass as bass
import concourse.tile as tile
import concourse.bass_utils as _bu
import concourse.bass2jax as _b2j
from concourse import mybir
from concourse.ap import AP
from concourse.bass_utils import run_bass_kernel_spmd
from bass_rust import ScopedClock, VectorClock

F32 = mybir.dt.float32
BF16 = mybir.dt.bfloat16
FP8 = mybir.dt.float8e4


def _patched_drain_and_barrier(self, tick_clock, wait_clock):
    ticks = list(tick_clock.global_clock)
    for idx, t in enumerate(ticks):
        if t > 0:
            single = [0] * len(ticks)
            single[idx] = t
            nop = self.nc.sync.nop(nofuse=True)
            wait_clock.add_sem_waits(nop.ins, ScopedClock({None: VectorClock(single)}))
    self.nc.sync.drain()
    popped = self.nc._tile_sem_poison_stack.pop()
    assert popped is self._sem_poison
    # no device-side clear_and_free + second barrier: the program epilogue
    # already resets the whole semaphore file before the NEFF completes


def _split_waits_in_bir(bir_bytes):
    bir = orjson.loads(bir_bytes)
    changed = False
    for fn in bir.get("functions", []):
        for blk in fn.get("blocks", []):
            out = []
            for inst in blk.get("instructions", []):
                si = inst.get("sync_info")
                waits = (si or {}).get("on_wait") or []
                if len(waits) > 1:
                    changed = True
                    for i, w in enumerate(waits[:-1]):
                        out.append({
                            "name": f"{inst['name']}-ws{i}",
                            "opcode": "NoOp",
                            "engine": inst.get("engine"),
                            "debug": inst.get("debug", 0),
                            "ins": [], "outs": [],
                            "sync_info": {"on_wait": [w], "on_update": []},
                        })
                    si["on_wait"] = [waits[-1]]
                out.append(inst)
            blk["instructions"] = out
    return orjson.dumps(bir) if changed else bir_bytes


_PATCHED = False


def _apply_patches():
    global _PATCHED
    if _PATCHED:
        return
    _PATCHED = True
    # If BASS_TRACE is set but the environment lacks antenv.axon_hooks,
    # run_bass_kernel_spmd would crash importing it; register a stub that
    # reports "no hook" so execution proceeds untraced.
    try:
        import antenv.axon_hooks  # noqa: F401
    except ImportError:
        import sys
        import types
        mod = types.ModuleType("antenv.axon_hooks")
        mod._hook = None
        mod.get_axon_ntff_profile_hook = lambda: mod._hook
        mod.set_axon_ntff_profile_hook = lambda h: setattr(mod, "_hook", h)
        sys.modules["antenv.axon_hooks"] = mod
    tile.TileContext._drain_and_barrier = _patched_drain_and_barrier
    orig = _bu.compile_bir_kernel

    def wrapped(bir_json, tmpdir, neff_name="file.neff"):
        if isinstance(bir_json, str):
            bir_json = bir_json.encode()
        return orig(_split_waits_in_bir(bir_json), tmpdir, neff_name=neff_name)

    _bu.compile_bir_kernel = wrapped
    _b2j.compile_bir_kernel = wrapped


# ---------------------------------------------------------------------------
# problem constants (hardcoded per contract)
# ---------------------------------------------------------------------------
NB, N, NCH = 32, 16384, 64
NCORES = 8
NB_CORE = NB // NCORES          # 4 sequences per core
P = 11                          # FIR taps, ODD (P = 2H+1)
H = (P - 1) // 2                # 5 -> blocks G_0..G_5
T = 512                         # PSUM bank: 512 f32 per partition
NQ = N // 2                     # output cols per phase per sequence
NT = NQ // T                    # matmul tiles per sequence
M2 = (N + P - 1) // 2           # input image cols (right context included)
M8 = NQ + 3                     # fp8 image cols (max shift 2 + 1 + T-1)
WSCALE = 128.0                  # tap pre-scale so fp8 blocks stay normal


# ---------------------------------------------------------------------------
# host-side prep
# ---------------------------------------------------------------------------
try:
    import ml_dtypes
    ml_bf16 = ml_dtypes.bfloat16
    ml_e4 = ml_dtypes.float8_e4m3
except ImportError:  # pragma: no cover
    import jax.numpy as jnp
    ml_bf16 = jnp.bfloat16
    ml_e4 = jnp.float8_e4m3


def _make_taps(A, B, C, D):
    A64, B64, C64 = A.astype(np.float64), B.astype(np.float64), C.astype(np.float64)
    V = np.empty((P, 64, 64), np.float64)
    Ak = np.eye(64)
    for m in range(P - 1):
        V[P - 2 - m] = C64 @ Ak @ B64
        Ak = Ak @ A64
    V[P - 1] = D.astype(np.float64)
    return V


def _make_blocks(V):
    """G_j [128, 128], rows = contraction [lo;hi], cols = out [even;odd]."""
    G = np.zeros((H + 1, 128, 128), np.float64)
    for j in range(H + 1):
        blk = G[j]
        blk[0:64, 0:64] = V[2 * j].T
        if 2 * j + 1 <= P - 1:
            blk[64:128, 0:64] = V[2 * j + 1].T
        if j >= 1:
            blk[0:64, 64:128] = V[2 * j - 1].T
        blk[64:128, 64:128] = V[2 * j].T
    return G * WSCALE


def _prep_images(u):
    """u [32,N,64] f32 -> polyphase images [32,128,M2] bf16 + fp8 copy."""
    upp = np.zeros((NB, N + P - 1, NCH), ml_bf16)
    upp[:, P - 1:, :] = u.astype(ml_bf16)
    img = np.empty((NB, 128, M2), ml_bf16)
    img[:, 0:64, :] = upp[:, 0::2].transpose(0, 2, 1)
    img[:, 64:128, :] = upp[:, 1::2].transpose(0, 2, 1)
    img8 = img[:, :, :M8].astype(ml_e4)
    return img, img8


K_TRANS = 32                     # x0 transient horizon: 0.58^32 ~ 1e-8


def _transient(x0f, A, C):
    """[32, K_TRANS, 64] f64: y_k += C A^k x0 correction terms."""
    A64, C64 = A.astype(np.float64), C.astype(np.float64)
    M = np.empty((K_TRANS, 64, 64))
    Ak = np.eye(64)
    for k in range(K_TRANS):
        M[k] = C64 @ Ak
        Ak = A64 @ Ak
    return np.einsum('bx,kyx->bky', x0f.astype(np.float64), M)


# ---------------------------------------------------------------------------
# device program
# ---------------------------------------------------------------------------
def _build_program():
    nc = bass.Bass()
    x_in = nc.dram_tensor("x", [NB_CORE, 128, M2], BF16, kind="ExternalInput")
    x8_in = nc.dram_tensor("x8", [NB_CORE, 128, M8], FP8, kind="ExternalInput")
    w01_in = nc.dram_tensor("w01", [128, 2, 128], FP8, kind="ExternalInput")
    w23_in = nc.dram_tensor("w23", [128, 2, 128], FP8, kind="ExternalInput")
    w4_in = nc.dram_tensor("w4", [128, 128], BF16, kind="ExternalInput")
    w5_in = nc.dram_tensor("w5", [128, 128], BF16, kind="ExternalInput")
    y_out = nc.dram_tensor("y", [NB_CORE, 128, NQ], BF16, kind="ExternalOutput")

    with tile.TileContext(nc) as tc, ExitStack() as ctx:
        wpool = ctx.enter_context(tc.tile_pool(name="w", bufs=1))
        ipool = ctx.enter_context(tc.tile_pool(name="img", bufs=2))
        fpool = ctx.enter_context(tc.tile_pool(name="f8", bufs=2))
        ppool = ctx.enter_context(tc.tile_pool(name="ps", bufs=8, space="PSUM"))
        opool = ctx.enter_context(tc.tile_pool(name="out", bufs=2))

        # chunk sizes (in tiles) per sequence: ramp up at the very start so
        # the first matmul's data lands fast, big chunks in steady state,
        # ramp down at the very end so the final output DMA drains fast.
        seq_sizes = [[1, 1, 2, 4, 4, 4], [4, 4, 4, 4], [4, 4, 4, 4],
                     [4, 4, 4, 2, 1, 1]]
        assert all(sum(s) == NT for s in seq_sizes)
        chunks = []
        for b in range(NB_CORE):
            t0 = 0
            for ct in seq_sizes[b]:
                chunks.append((b, t0, ct))
                t0 += ct

        w01 = wpool.tile([128, 2, 128], FP8)
        w23 = wpool.tile([128, 2, 128], FP8)
        w4 = wpool.tile([128, 128], BF16)
        w5 = wpool.tile([128, 128], BF16)

        # halo: bf16 passes read shifts 4,5 -> +5 cols; fp8 reads +3.
        imgs = [ipool.tile([128, ct * T + 5], BF16, tag=f"img{ct}",
                           name=f"img_{k}", bufs=2)
                for k, (b, t0, ct) in enumerate(chunks)]
        f8s = [fpool.tile([128, ct * T + 3], FP8, tag=f"f8_{ct}",
                          name=f"f8_{k}", bufs=2)
               for k, (b, t0, ct) in enumerate(chunks)]

        # head: w4 FIRST (the warmup matmul depends only on it).  bf16 image
        # on the sync queue, fp8 image on the gpsimd queue -- except the
        # first two fp8 chunks, which ride sync because the gpsimd queue has
        # a ~10us cold start.  Output on the scalar queue.
        nc.sync.dma_start(w4[:], w4_in[:])
        nc.sync.dma_start(w5[:], w5_in[:])
        nc.gpsimd.dma_start(w01[:], w01_in[:])
        nc.gpsimd.dma_start(w23[:], w23_in[:])
        for k, (b, t0, ct) in enumerate(chunks):
            nc.sync.dma_start(imgs[k][:], x_in[b][:, t0 * T:t0 * T + ct * T + 5])
            nc.gpsimd.dma_start(f8s[k][:],
                                x8_in[b][:, t0 * T:t0 * T + ct * T + 3])

        # warmup: the first PE instruction must be a non-DoubleRow matmul
        # (a leading DR LDWEIGHTS/MATMUL wedges the device).  Shares the
        # "ps" tag so all 8 PSUM banks serve the main loop.
        with tc.high_priority():
            ps_warm = ppool.tile([128, T], F32, tag="ps", name="ps_warm",
                                 bufs=8)
            nc.tensor.matmul(ps_warm[:, 0:128], w4[:], w4[:], start=True,
                             stop=True, tile_position=(0, 0))

        def dr_rhs(f8t, col0):
            ap = f8t[:]
            pstride = ap.ap[0][0]
            return AP(tensor=ap.tensor, offset=ap.offset + col0,
                      ap=[[pstride, 128], [1, 2], [1, T]])

        tile_idx = 0
        GRP = 4          # tiles per mode-batch: DR x2G then bf16 x2G --
        #                  amortizes the PE DR<->bf16 mode-switch penalty
        for k, (b, t0, ct) in enumerate(chunks):
            img = imgs[k]
            f8 = f8s[k]
            c0 = t0 * T
            yc = opool.tile([128, ct * T], BF16, tag=f"out{ct}")
            for g0 in range(0, ct, GRP):
                gts = list(range(g0, min(g0 + GRP, ct)))
                pss = {t: ppool.tile([128, T], F32, bufs=8, tag="ps",
                                     name=f"ps_{k}_{t}") for t in gts}
                for t in gts:
                    s0 = t * T
                    nc.tensor.matmul(pss[t][:], w01[:], dr_rhs(f8, s0),
                                     start=True, stop=False,
                                     perf_mode=mybir.MatmulPerfMode.DoubleRow,
                                     tile_position=(0, 0),
                                     skip_group_check=True)
                    nc.tensor.matmul(pss[t][:], w23[:], dr_rhs(f8, s0 + 2),
                                     start=False, stop=False,
                                     perf_mode=mybir.MatmulPerfMode.DoubleRow,
                                     tile_position=(0, 0),
                                     skip_group_check=True)
                for t in gts:
                    s0 = t * T
                    nc.tensor.matmul(pss[t][:], w4[:],
                                     img[:, s0 + 4:s0 + 4 + T],
                                     start=False, stop=False,
                                     tile_position=(0, 0),
                                     skip_group_check=True)
                    nc.tensor.matmul(pss[t][:], w5[:],
                                     img[:, s0 + 5:s0 + 5 + T],
                                     start=False, stop=True,
                                     tile_position=(0, 0),
                                     skip_group_check=True)
                    # evacuate with the 1/WSCALE fold-in, alternating engines
                    if tile_idx % 2 == 0:
                        nc.vector.tensor_scalar_mul(yc[:, s0:s0 + T],
                                                    pss[t][:], 1.0 / WSCALE)
                    else:
                        nc.scalar.mul(yc[:, s0:s0 + T], pss[t][:],
                                      1.0 / WSCALE)
                    tile_idx += 1
                # ship each group's columns as soon as its evacs land --
                # smooths the output queue and cuts the end-of-run backlog
                gc0 = c0 + g0 * T
                gw = (gts[-1] + 1 - g0) * T
                nc.scalar.dma_start(y_out[b][:, gc0:gc0 + gw],
                                    yc[:, g0 * T:g0 * T + gw])

    # HW safety check: the first tensor-engine matmul/ldweights must not be
    # DoubleRow (see module docstring).
    for blk in nc.m.functions[0].blocks:
        for ins in blk.instructions:
            nm = type(ins).__name__
            if nm in ("InstMatmult", "InstLdweights"):
                assert getattr(ins, "perf_mode", None) is None, (
                    "first PE instruction is DoubleRow -- would wedge the HW")
                return nc
    return nc


_PROGRAM = None
_LAST_RESULTS = None


def kernel(x0, us, A, B, C, D):
    _apply_patches()
    global _PROGRAM, _LAST_RESULTS
    if _PROGRAM is None:
        _PROGRAM = _build_program()

    x0 = np.asarray(x0, np.float32)
    us = np.asarray(us, np.float32)
    u = us[..., 0]                      # [32, N, 64]
    x0f = x0[..., 0]                    # [32, 64]

    V = _make_taps(np.asarray(A), np.asarray(B), np.asarray(C), np.asarray(D))
    G = _make_blocks(V)                 # f64, pre-scaled by WSCALE
    w01 = np.stack([G[0], G[1]], axis=1).astype(ml_e4)    # [128, 2, 128]
    w23 = np.stack([G[2], G[3]], axis=1).astype(ml_e4)
    w4 = G[4].astype(ml_bf16)
    w5 = G[5].astype(ml_bf16)

    imgs, imgs8 = _prep_images(u)

    in_maps = []
    for c in range(NCORES):
        sl = slice(c * NB_CORE, (c + 1) * NB_CORE)
        in_maps.append({"x": np.ascontiguousarray(imgs[sl]),
                        "x8": np.ascontiguousarray(imgs8[sl]),
                        "w01": w01, "w23": w23, "w4": w4, "w5": w5})

    res = run_bass_kernel_spmd(_PROGRAM, in_maps, list(range(NCORES)))
    _LAST_RESULTS = res

    out = np.empty((NB, 128, NQ), ml_bf16)
    for c in range(NCORES):
        out[c * NB_CORE:(c + 1) * NB_CORE] = np.asarray(res.results[c]["y"])
    y = (out.reshape(NB, 2, 64, NQ)
            .transpose(0, 3, 1, 2)
            .reshape(NB, N, 64)
            .astype(np.float32))
    y[:, :K_TRANS, :] += _transient(x0f, np.asarray(A), np.asarray(C)).astype(
        np.float32)
    return y[..., None]


# revision 33
# speedup vs baseline: 1.0568x; 1.0568x over previous
"""Trainium2 Bass kernel for the batched linear state-space model

    x_{k+1} = A x_k + B u_k ;  y_k = C x_k + D u_k   (y uses pre-update state)

Shapes: x0 [32,64,1], us [32,16384,64,1], A/B/C/D [64,64] -> y [32,16384,64,1].

Method (v3: fp8 DoubleRow tail taps)
------------------------------------
A is stable (spectral radius ~0.58), so the exact scan equals a causal FIR
with geometrically decaying matrix taps, truncated at P = 11 taps:

    y_k = sum_{i=0}^{P-1} V_i u''[k+i],   V_i = C A^{P-2-i} B (i<P-1), V_{P-1}=D

with u'' = [0...0, u_0 ... u_{N-1}] (P-1 zeros prepended).  The x0
transient sum_k C A^k x0 is added EXACTLY on the host for k < 32.

Polyphase S=2 layout as in v2: SBUF partitions 0:64 hold lo[m] = u''[2m],
64:128 hold hi[m] = u''[2m+1]; block G_j (128x128) at shift j combines taps
{V_{2j-1}, V_{2j}, V_{2j+1}} so that one [128,T] rhs pass yields T even and
T odd outputs.  P=11 gives 6 blocks G_0..G_5.

New in v3: G_0..G_3 carry only small taps (|CA^kB| ~ 0.58^k, k>=2), so they
run in fp8 e4m3 with MatmulPerfMode.DoubleRow: one PE pass contracts 256
rows (two 128-blocks at adjacent shifts) per column -- 2x MACs/cycle.  The
hardware accepts rhs APs whose two K-halves are the SAME fp8 image at
column shifts (j, j+1) (dim1 stride 1, verified on HW).  Dominant blocks
G_4, G_5 (taps D, CB, CAB, CA^2B) stay bf16.  Per 512-col tile: 2 bf16
passes + 2 DR passes = 4 PE passes vs 5 in v2 (and truncation error drops:
P 9 -> 11).  All taps are pre-scaled by 128 so the smallest fp8 blocks stay
in e4m3's normal range; the PSUM evacuation multiplies by 1/128 while
converting f32 -> bf16 (vector tensor_scalar / scalar activation, split
between both engines).

HW-verified constraints baked in here (each wedges the device if violated):
  * the FIRST tensor-engine instruction must be a non-DoubleRow matmul
    (hence the warmup matmul, forced first via high_priority + first DMA);
  * within one PSUM accumulation group, DR passes must come BEFORE bf16
    passes (bf16 -> DR transition with start=False is fatal);
  * no engine may WRITE fp8 to SBUF in a program containing DR matmuls
    (vector/scalar/gpsimd casts all crash), so the fp8 image is prepared on
    the HOST and DMA'd in (gpsimd queue) alongside the bf16 image (sync
    queue); output goes out on the scalar queue.  Per-core DMA: 12.6 MB in
    + 8.4 MB out.  DMA chunk sizes ramp 1,1,2,4,8 tiles at the start and
    down at the end as in v2.
"""
import numpy as np
from contextlib import ExitStack

# ---------------------------------------------------------------------------
# environment patches (this container's walrus encodes at most ONE semaphore
# wait per instruction; Tile emits more on the exit drain and on join points)
# ---------------------------------------------------------------------------
import orjson
import concourse.bass as bass
import concourse.tile as tile
import concourse.bass_utils as _bu
import concourse.bass2jax as _b2j
from concourse import mybir
from concourse.ap import AP
from concourse.bass_utils import run_bass_kernel_spmd
from bass_rust import ScopedClock, VectorClock

F32 = mybir.dt.float32
BF16 = mybir.dt.bfloat16
FP8 = mybir.dt.float8e4


def _patched_drain_and_barrier(self, tick_clock, wait_clock):
    ticks = list(tick_clock.global_clock)
    for idx, t in enumerate(ticks):
        if t > 0:
            single = [0] * len(ticks)
            single[idx] = t
            nop = self.nc.sync.nop(nofuse=True)
            wait_clock.add_sem_waits(nop.ins, ScopedClock({None: VectorClock(single)}))
    self.nc.sync.drain()
    popped = self.nc._tile_sem_poison_stack.pop()
    assert popped is self._sem_poison
    # no device-side clear_and_free + second barrier: the program epilogue
    # already resets the whole semaphore file before the NEFF completes


def _split_waits_in_bir(bir_bytes):
    bir = orjson.loads(bir_bytes)
    changed = False
    for fn in bir.get("functions", []):
        for blk in fn.get("blocks", []):
            out = []
            for inst in blk.get("instructions", []):
                si = inst.get("sync_info")
                waits = (si or {}).get("on_wait") or []
                if len(waits) > 1:
                    changed = True
                    for i, w in enumerate(waits[:-1]):
                        out.append({
                            "name": f"{inst['name']}-ws{i}",
                            "opcode": "NoOp",
                            "engine": inst.get("engine"),
                            "debug": inst.get("debug", 0),
                            "ins": [], "outs": [],
                            "sync_info": {"on_wait": [w], "on_update": []},
                        })
                    si["on_wait"] = [waits[-1]]
                out.append(inst)
            blk["instructions"] = out
    return orjson.dumps(bir) if changed else bir_bytes


_PATCHED = False


def _apply_patches():
    global _PATCHED
    if _PATCHED:
        return
    _PATCHED = True
    # If BASS_TRACE is set but the environment lacks antenv.axon_hooks,
    # run_bass_kernel_spmd would crash importing it; register a stub that
    # reports "no hook" so execution proceeds untraced.
    try:
        import antenv.axon_hooks  # noqa: F401
    except ImportError:
        import sys
        import types
        mod = types.ModuleType("antenv.axon_hooks")
        mod._hook = None
        mod.get_axon_ntff_profile_hook = lambda: mod._hook
        mod.set_axon_ntff_profile_hook = lambda h: setattr(mod, "_hook", h)
        sys.modules["antenv.axon_hooks"] = mod
    tile.TileContext._drain_and_barrier = _patched_drain_and_barrier
    orig = _bu.compile_bir_kernel

    def wrapped(bir_json, tmpdir, neff_name="file.neff"):
        if isinstance(bir_json, str):
            bir_json = bir_json.encode()
        return orig(_split_waits_in_bir(bir_json), tmpdir, neff_name=neff_name)

    _bu.compile_bir_kernel = wrapped
    _b2j.compile_bir_kernel = wrapped


# ---------------------------------------------------------------------------
# problem constants (hardcoded per contract)
# ---------------------------------------------------------------------------
NB, N, NCH = 32, 16384, 64
NCORES = 8
NB_CORE = NB // NCORES          # 4 sequences per core
P = 11                          # FIR taps, ODD (P = 2H+1)
H = (P - 1) // 2                # 5 -> blocks G_0..G_5
T = 512                         # PSUM bank: 512 f32 per partition
NQ = N // 2                     # output cols per phase per sequence
NT = NQ // T                    # matmul tiles per sequence
M2 = (N + P - 1) // 2           # input image cols (right context included)
M8 = NQ + 3                     # fp8 image cols (max shift 2 + 1 + T-1)
WSCALE = 128.0                  # tap pre-scale so fp8 blocks stay normal


# ---------------------------------------------------------------------------
# host-side prep
# ---------------------------------------------------------------------------
try:
    import ml_dtypes
    ml_bf16 = ml_dtypes.bfloat16
    ml_e4 = ml_dtypes.float8_e4m3
except ImportError:  # pragma: no cover
    import jax.numpy as jnp
    ml_bf16 = jnp.bfloat16
    ml_e4 = jnp.float8_e4m3


def _make_taps(A, B, C, D):
    A64, B64, C64 = A.astype(np.float64), B.astype(np.float64), C.astype(np.float64)
    V = np.empty((P, 64, 64), np.float64)
    Ak = np.eye(64)
    for m in range(P - 1):
        V[P - 2 - m] = C64 @ Ak @ B64
        Ak = Ak @ A64
    V[P - 1] = D.astype(np.float64)
    return V


def _make_blocks(V):
    """G_j [128, 128], rows = contraction [lo;hi], cols = out [even;odd]."""
    G = np.zeros((H + 1, 128, 128), np.float64)
    for j in range(H + 1):
        blk = G[j]
        blk[0:64, 0:64] = V[2 * j].T
        if 2 * j + 1 <= P - 1:
            blk[64:128, 0:64] = V[2 * j + 1].T
        if j >= 1:
            blk[0:64, 64:128] = V[2 * j - 1].T
        blk[64:128, 64:128] = V[2 * j].T
    return G * WSCALE


def _prep_images(u):
    """u [32,N,64] f32 -> polyphase images [32,128,M2] bf16 + fp8 copy."""
    upp = np.zeros((NB, N + P - 1, NCH), ml_bf16)
    upp[:, P - 1:, :] = u.astype(ml_bf16)
    img = np.empty((NB, 128, M2), ml_bf16)
    img[:, 0:64, :] = upp[:, 0::2].transpose(0, 2, 1)
    img[:, 64:128, :] = upp[:, 1::2].transpose(0, 2, 1)
    img8 = img[:, :, :M8].astype(ml_e4)
    return img, img8


K_TRANS = 32                     # x0 transient horizon: 0.58^32 ~ 1e-8


def _transient(x0f, A, C):
    """[32, K_TRANS, 64] f64: y_k += C A^k x0 correction terms."""
    A64, C64 = A.astype(np.float64), C.astype(np.float64)
    M = np.empty((K_TRANS, 64, 64))
    Ak = np.eye(64)
    for k in range(K_TRANS):
        M[k] = C64 @ Ak
        Ak = A64 @ Ak
    return np.einsum('bx,kyx->bky', x0f.astype(np.float64), M)


# ---------------------------------------------------------------------------
# device program
# ---------------------------------------------------------------------------
def _build_program():
    nc = bass.Bass()
    x_in = nc.dram_tensor("x", [NB_CORE, 128, M2], BF16, kind="ExternalInput")
    x8_in = nc.dram_tensor("x8", [NB_CORE, 128, M8], FP8, kind="ExternalInput")
    w01_in = nc.dram_tensor("w01", [128, 2, 128], FP8, kind="ExternalInput")
    w23_in = nc.dram_tensor("w23", [128, 2, 128], FP8, kind="ExternalInput")
    w4_in = nc.dram_tensor("w4", [128, 128], BF16, kind="ExternalInput")
    w5_in = nc.dram_tensor("w5", [128, 128], BF16, kind="ExternalInput")
    y_out = nc.dram_tensor("y", [NB_CORE, 128, NQ], BF16, kind="ExternalOutput")

    with tile.TileContext(nc) as tc, ExitStack() as ctx:
        wpool = ctx.enter_context(tc.tile_pool(name="w", bufs=1))
        ipool = ctx.enter_context(tc.tile_pool(name="img", bufs=2))
        fpool = ctx.enter_context(tc.tile_pool(name="f8", bufs=2))
        ppool = ctx.enter_context(tc.tile_pool(name="ps", bufs=8, space="PSUM"))
        opool = ctx.enter_context(tc.tile_pool(name="out", bufs=2))

        # chunk sizes (in tiles) per sequence: ramp up at the very start so
        # the first matmul's data lands fast, big chunks in steady state,
        # ramp down at the very end so the final output DMA drains fast.
        seq_sizes = [[1, 1, 2, 4, 8], [8, 8], [8, 8], [8, 4, 2, 1, 1]]
        assert all(sum(s) == NT for s in seq_sizes)
        chunks = []
        for b in range(NB_CORE):
            t0 = 0
            for ct in seq_sizes[b]:
                chunks.append((b, t0, ct))
                t0 += ct

        w01 = wpool.tile([128, 2, 128], FP8)
        w23 = wpool.tile([128, 2, 128], FP8)
        w4 = wpool.tile([128, 128], BF16)
        w5 = wpool.tile([128, 128], BF16)

        # halo: bf16 passes read shifts 4,5 -> +5 cols; fp8 reads +3.
        imgs = [ipool.tile([128, ct * T + 5], BF16, tag=f"img{ct}",
                           name=f"img_{k}", bufs=2)
                for k, (b, t0, ct) in enumerate(chunks)]
        f8s = [fpool.tile([128, ct * T + 3], FP8, tag=f"f8_{ct}",
                          name=f"f8_{k}", bufs=2)
               for k, (b, t0, ct) in enumerate(chunks)]

        # head: w4 FIRST (the warmup matmul depends only on it).  bf16 image
        # on the sync queue, fp8 image on the gpsimd queue -- except the
        # first two fp8 chunks, which ride sync because the gpsimd queue has
        # a ~10us cold start.  Output on the scalar queue.
        nc.sync.dma_start(w4[:], w4_in[:])
        nc.sync.dma_start(w5[:], w5_in[:])
        nc.gpsimd.dma_start(w01[:], w01_in[:])
        nc.gpsimd.dma_start(w23[:], w23_in[:])
        for k, (b, t0, ct) in enumerate(chunks):
            nc.sync.dma_start(imgs[k][:], x_in[b][:, t0 * T:t0 * T + ct * T + 5])
            nc.gpsimd.dma_start(f8s[k][:],
                                x8_in[b][:, t0 * T:t0 * T + ct * T + 3])

        # warmup: the first PE instruction must be a non-DoubleRow matmul
        # (a leading DR LDWEIGHTS/MATMUL wedges the device).  Shares the
        # "ps" tag so all 8 PSUM banks serve the main loop.
        with tc.high_priority():
            ps_warm = ppool.tile([128, T], F32, tag="ps", name="ps_warm",
                                 bufs=8)
            # ~24 back-to-back dummy passes keep the PE clock ramping
            # through the DMA cold-start window (it idles otherwise and the
            # first real tiles would run at the low-pstate half clock);
            # they chain on the same PSUM region and finish before the
            # first image chunk lands.
            for _ in range(24):
                nc.tensor.matmul(ps_warm[:, 0:128], w4[:], w4[:], start=True,
                                 stop=True, tile_position=(0, 0))

        def dr_rhs(f8t, col0):
            ap = f8t[:]
            pstride = ap.ap[0][0]
            return AP(tensor=ap.tensor, offset=ap.offset + col0,
                      ap=[[pstride, 128], [1, 2], [1, T]])

        tile_idx = 0
        GRP = 4          # tiles per mode-batch: DR x2G then bf16 x2G --
        #                  amortizes the PE DR<->bf16 mode-switch penalty
        for k, (b, t0, ct) in enumerate(chunks):
            img = imgs[k]
            f8 = f8s[k]
            c0 = t0 * T
            yc = opool.tile([128, ct * T], BF16, tag=f"out{ct}")
            for g0 in range(0, ct, GRP):
                gts = list(range(g0, min(g0 + GRP, ct)))
                pss = {t: ppool.tile([128, T], F32, bufs=8, tag="ps",
                                     name=f"ps_{k}_{t}") for t in gts}
                for t in gts:
                    s0 = t * T
                    nc.tensor.matmul(pss[t][:], w01[:], dr_rhs(f8, s0),
                                     start=True, stop=False,
                                     perf_mode=mybir.MatmulPerfMode.DoubleRow,
                                     tile_position=(0, 0),
                                     skip_group_check=True)
                    nc.tensor.matmul(pss[t][:], w23[:], dr_rhs(f8, s0 + 2),
                                     start=False, stop=False,
                                     perf_mode=mybir.MatmulPerfMode.DoubleRow,
                                     tile_position=(0, 0),
                                     skip_group_check=True)
                for t in gts:
                    s0 = t * T
                    nc.tensor.matmul(pss[t][:], w4[:],
                                     img[:, s0 + 4:s0 + 4 + T],
                                     start=False, stop=False,
                                     tile_position=(0, 0),
                                     skip_group_check=True)
                    nc.tensor.matmul(pss[t][:], w5[:],
                                     img[:, s0 + 5:s0 + 5 + T],
                                     start=False, stop=True,
                                     tile_position=(0, 0),
                                     skip_group_check=True)
                    # evacuate with the 1/WSCALE fold-in, alternating engines
                    if tile_idx % 2 == 0:
                        nc.vector.tensor_scalar_mul(yc[:, s0:s0 + T],
                                                    pss[t][:], 1.0 / WSCALE)
                    else:
                        nc.scalar.mul(yc[:, s0:s0 + T], pss[t][:],
                                      1.0 / WSCALE)
                    tile_idx += 1
                # ship each group's columns as soon as its evacs land --
                # smooths the output queue and cuts the end-of-run backlog
                gc0 = c0 + g0 * T
                gw = (gts[-1] + 1 - g0) * T
                nc.scalar.dma_start(y_out[b][:, gc0:gc0 + gw],
                                    yc[:, g0 * T:g0 * T + gw])

    # HW safety check: the first tensor-engine matmul/ldweights must not be
    # DoubleRow (see module docstring).
    for blk in nc.m.functions[0].blocks:
        for ins in blk.instructions:
            nm = type(ins).__name__
            if nm in ("InstMatmult", "InstLdweights"):
                assert getattr(ins, "perf_mode", None) is None, (
                    "first PE instruction is DoubleRow -- would wedge the HW")
                return nc
    return nc


_PROGRAM = None
_LAST_RESULTS = None


def kernel(x0, us, A, B, C, D):
    _apply_patches()
    global _PROGRAM, _LAST_RESULTS
    if _PROGRAM is None:
        _PROGRAM = _build_program()

    x0 = np.asarray(x0, np.float32)
    us = np.asarray(us, np.float32)
    u = us[..., 0]                      # [32, N, 64]
    x0f = x0[..., 0]                    # [32, 64]

    V = _make_taps(np.asarray(A), np.asarray(B), np.asarray(C), np.asarray(D))
    G = _make_blocks(V)                 # f64, pre-scaled by WSCALE
    w01 = np.stack([G[0], G[1]], axis=1).astype(ml_e4)    # [128, 2, 128]
    w23 = np.stack([G[2], G[3]], axis=1).astype(ml_e4)
    w4 = G[4].astype(ml_bf16)
    w5 = G[5].astype(ml_bf16)

    imgs, imgs8 = _prep_images(u)

    in_maps = []
    for c in range(NCORES):
        sl = slice(c * NB_CORE, (c + 1) * NB_CORE)
        in_maps.append({"x": np.ascontiguousarray(imgs[sl]),
                        "x8": np.ascontiguousarray(imgs8[sl]),
                        "w01": w01, "w23": w23, "w4": w4, "w5": w5})

    res = run_bass_kernel_spmd(_PROGRAM, in_maps, list(range(NCORES)))
    _LAST_RESULTS = res

    out = np.empty((NB, 128, NQ), ml_bf16)
    for c in range(NCORES):
        out[c * NB_CORE:(c + 1) * NB_CORE] = np.asarray(res.results[c]["y"])
    y = (out.reshape(NB, 2, 64, NQ)
            .transpose(0, 3, 1, 2)
            .reshape(NB, N, 64)
            .astype(np.float32))
    y[:, :K_TRANS, :] += _transient(x0f, np.asarray(A), np.asarray(C)).astype(
        np.float32)
    return y[..., None]
